# revision 53
# baseline (speedup 1.0000x reference)
"""Trainium2 Bass kernel for gated dense attention with dim=0 softmax.

Computation (reference):
    h = x @ W1 + b1
    q,k,v = h @ W{q,k,v} + b{q,k,v}
    w = (q @ k.T) / sqrt(256)
    attn = softmax(w, axis=0)          # normalizes over ROWS per column
    h2 = a*h + (1-a)*(attn @ v)
    out = h2 @ W2 + b2

Distribution strategy (chosen for the cost model's collective pricing:
every collective costs a flat ~15us + bytes/40GBps, AllReduce x1.875):

  Replicate x to all 8 cores as fp8 (4MB HBM->SBUF stream at 360GB/s is
  far cheaper than any AllGather at collective rates).  Core r owns
  COLUMN block J_r = [r*1024, (r+1)*1024) of the attention matrix: it
  computes q for ALL rows locally (x replicated), k,v only for its own
  rows, then S^r = k_r @ q_all^T -> [1024 j, 8192 i].  The dim=0
  softmax denominator sums over ALL i -- fully local in this layout (no
  stats AllReduce).  Each core forms the partial output
  u^r[i,:] = sum_{j in J_r} P[i,j] v'[j,:] for ALL i, and a single fp8
  ReduceScatter (256KB out, ~21.5us; reduction itself runs fp32) both
  sums the partials over cores and hands each core its own 1024 rows.

  Collectives: ONE ReduceScatter.  (Baseline: AG-k + AG-v + 4 stats
  AllReduces ~ 249us of serialized collective time.)

Schedule: the pacing engine is ScalarE doing the 8.4M exps/core.  exp
runs at [128,2048] grain from a 2-deep psum ping-pong, back-to-back at
1892ns; S matmuls (fp8 DoubleRow, ~1.8us/chunk) hide underneath, and
q-chunk production is interleaved INTO the S stream so the first exp
fires ~11us in.  Column sums run on the otherwise-idle Pool engine
(tensor_reduce over the just-written fp8 P tiles) except the last
i-chunk, which uses the exp's accum_out so the per-j scale is ready
the moment its final exp retires.  During the ReduceScatter the PE
computes the residual x@(a*W1W2) term and then chews dummy f32 matmuls
to stay at full clock for the post-collective projection.

Weight folding (host, O(weights) only): q = x @ (W1@Wq) + (b1@Wq + bq),
same for k,v; y = x @ (a*W1@W2) + u'' @ (8*W2) / (8*VS) + (a*b1@W2+b2)
with u'' = VS*(1-a)*u carried through the fp8 ReduceScatter.
"""

import numpy as np
import ml_dtypes

BF16 = ml_dtypes.bfloat16
FP8 = ml_dtypes.float8_e4m3fn

N, D, H, C = 8192, 512, 256, 256
NCORES = 8
NL = N // NCORES          # 1024 rows/columns per core
JT = NL // 128            # 8 local j-tiles
NPAIR = JT // 2           # 4 DoubleRow j-pairs
NIC = N // 2048           # 4 i-chunks at the 2048-wide exp grain
WS = 16.0                 # fp8 range scale on folded Wq/Wk
VS = 4096.0               # range shim on v' and u'' (u'' stays in fp8 range)
W2S = 8.0                 # fp8 range scale on W2
EXPSCALE = 1.0 / (WS * WS * 16.0)   # restores exp(q.k/sqrt(256))
NBP = 10 + 2 * C          # packed bias columns (gsc, bv, b12, gb x4, cb)
NDUMMY = 30               # f32 warm-up matmuls spanning the ReduceScatter

_CACHED = {}


def _build():
    import concourse.mybir as mybir
    from concourse import bacc
    from concourse.tile import TileContext

    dt = mybir.dt
    AF = mybir.ActivationFunctionType
    DR = mybir.MatmulPerfMode.DoubleRow
    f32, bf, f8 = dt.float32, dt.bfloat16, dt.float8e4
    RG = [list(range(NCORES))]

    nc = bacc.Bacc(None, target_bir_lowering=False, num_devices=NCORES)

    # ---------------- I/O (per core) ----------------
    xT8 = nc.declare_dram_parameter("xT8", [128, 4, N], f8, isOutput=False)
    xTl8 = nc.declare_dram_parameter("xTl8", [128, 4, NL], f8, isOutput=False)
    xTlb = nc.declare_dram_parameter("xTlb", [128, 4, NL], bf, isOutput=False)
    g8 = nc.declare_dram_parameter("g8", [128, 4, D], f8, isOutput=False)
    wb8 = nc.declare_dram_parameter("wb8", [128, 4, 1], f8, isOutput=False)
    wv8 = nc.declare_dram_parameter("wv8", [128, 4, H], f8, isOutput=False)
    w12b = nc.declare_dram_parameter("w12b", [128, 4, C], bf, isOutput=False)
    w2f = nc.declare_dram_parameter("w2f", [128, 2, C], f8, isOutput=False)
    bpk = nc.declare_dram_parameter("bpk", [128, NBP], f32, isOutput=False)
    y = nc.declare_dram_parameter("y", [NL, C], f32, isOutput=True)

    # partial outputs, laid out [rank][cp, ct, i'] so the ReduceScatter
    # shard for rank r is u''^T for its own rows, lhsT-ready
    pbuf = nc.dram_tensor("pbuf", [NCORES * 2 * 128 * NL], f8)
    rsout = nc.dram_tensor("rsout", [2 * 128 * NL], f8)

    with TileContext(nc) as tc:
        with (
            tc.tile_pool(name="cst", bufs=1) as cst,
            tc.tile_pool(name="big", bufs=1) as big,
            tc.tile_pool(name="strm", bufs=1) as strm,
            tc.tile_pool(name="psum", bufs=1, space="PSUM") as psum,
        ):
            # ---- SBUF residents ----
            bpt = cst.tile([128, NBP], f32, tag="bpt", name="bpt")
            g8t = cst.tile([128, 4, D], f8, tag="g8t", name="g8t")
            wb8t = cst.tile([128, 4, 1], f8, tag="wb8t", name="wb8t")
            wvt = cst.tile([128, 4, H], f8, tag="wvt", name="wvt")
            w12t = cst.tile([128, 4, C], bf, tag="w12t", name="w12t")
            w2t = cst.tile([128, 2, C], f8, tag="w2t", name="w2t")
            scr = cst.tile([128, 1], f32, tag="scr", name="scr")

            xt8 = big.tile([128, 4, N], f8, tag="xt8", name="xt8")
            xtl8 = big.tile([128, 4, NL], f8, tag="xtl8", name="xtl8")
            xtlb = big.tile([128, 4, NL], bf, tag="xtlb", name="xtlb")
            mts = big.tile([128, 4, NL], f8, tag="mts", name="mts")
            bqm = big.tile([128, JT], f32, tag="bqm", name="bqm")
            vts = [big.tile([128, 2, C], f8, tag=f"vts{t}", name=f"vts{t}")
                   for t in range(NPAIR)]
            vss = [big.tile([128, 2, C], f8, tag=f"vss{t}", name=f"vss{t}")
                   for t in range(NPAIR)]
            pts = [big.tile([128, 2, N], f8, tag=f"pts{t}", name=f"pts{t}")
                   for t in range(NPAIR)]
            colsq = big.tile([128, 4 * JT], f32, tag="colsq", name="colsq")
            csum = big.tile([128, JT], f32, tag="csum", name="csum")
            ginv = big.tile([128, JT], f32, tag="ginv", name="ginv")
            uts = big.tile([128, 2, NL], f8, tag="uts", name="uts")
            xyt = big.tile([128, 8, C], f32, tag="xyt", name="xyt")

            gsc = bpt[:, 4:5]
            bvt = bpt[:, 5:5 + C]
            b2t = bpt[:, 5 + C:5 + 2 * C]
            gbc = lambda t: bpt[:, 5 + 2 * C + t:6 + 2 * C + t]
            cbc = bpt[:, 9 + 2 * C:10 + 2 * C]

            # ---- ACT table preload: Exp table load off the critical path
            nc.vector.memset(scr, 0.0)
            nc.scalar.activation(scr, scr, AF.Exp, scale=0.0)

            # ---- input DMAs ----
            nc.scalar.dma_start(out=g8t, in_=g8[:])
            nc.scalar.dma_start(out=wb8t, in_=wb8[:])
            nc.scalar.dma_start(out=bpt, in_=bpk[:])
            nc.gpsimd.dma_start(out=xtl8, in_=xTl8[:])
            nc.gpsimd.dma_start(out=wvt, in_=wv8[:])
            for c in range(NIC):
                sl = slice(c * 2048, (c + 1) * 2048)
                nc.sync.dma_start(out=xt8[:, :, sl], in_=xT8[:, :, sl])
            nc.sync.dma_start(out=xtlb, in_=xTlb[:])
            nc.sync.dma_start(out=w12t, in_=w12b[:])
            nc.sync.dma_start(out=w2t, in_=w2f[:])

            # ---- head: M^T = G @ x_l^T + gb where G = 256*(Wq' Wk'^T) is
            # host-folded, so S contracts M directly against the replicated
            # x (neither q nor k ever materializes; the q-bias row enters
            # via the exp's per-partition bias, itself a folded matvec).
            # exp-bias matvec first: bias_j = x_l[j].(Wk' bq)/16 + bk.bq/16
            bqp = psum.tile([128, 2048], f32, tag="S", bufs=2, name="bqp")
            for jt in range(JT):
                for u in range(2):
                    nc.tensor.matmul(
                        bqp[:, jt:jt + 1],
                        lhsT=xtl8[:, 2 * u:2 * u + 2, jt * 128:(jt + 1) * 128],
                        rhs=wb8t[:, 2 * u:2 * u + 2, :],
                        start=(u == 0), stop=(u == 1), perf_mode=DR,
                    )
            mp = [psum.tile([128, 2048], f32, tag="S", bufs=2, name=f"mp{h}")
                  for h in range(2)]
            for hk in range(4):
                for jc in range(2):
                    for u in range(2):
                        nc.tensor.matmul(
                            mp[hk // 2][:, (hk % 2) * 1024 + jc * 512:
                                        (hk % 2) * 1024 + (jc + 1) * 512],
                            lhsT=g8t[:, 2 * u:2 * u + 2, hk * 128:(hk + 1) * 128],
                            rhs=xtl8[:, 2 * u:2 * u + 2, jc * 512:(jc + 1) * 512],
                            start=(u == 0), stop=(u == 1), perf_mode=DR,
                        )
            nc.vector.tensor_scalar(
                bqm, bqp[:, 0:JT], 1.0 / 1024.0, cbc,
                op0=mybir.AluOpType.mult, op1=mybir.AluOpType.add)
            # three-way split so the last M quarter lands ~1us earlier
            for h in range(2):
                for e2 in range(2):
                    hq = 2 * h + e2
                    src = mp[h][:, e2 * 1024:(e2 + 1) * 1024]
                    if hq == 2:
                        nc.scalar.activation(mts[:, hq, :], src, AF.Identity,
                                             bias=gbc(hq))
                    elif hq == 0:
                        nc.vector.tensor_scalar_add(mts[:, hq, :], src,
                                                    gbc(hq))
                    else:
                        nc.gpsimd.tensor_scalar_add(mts[:, hq, :], src,
                                                    gbc(hq))

            def v_mms(lo):
                # v[j_local, c] in halves slotted under the exp pipeline's
                # PE slack (a full 16-matmul burst would starve one exp)
                vp = psum.tile([128, 2048], f32, tag="S", bufs=2, name="vp")
                for jt in range(lo, lo + 4):
                    for u in range(2):
                        nc.tensor.matmul(
                            vp[:, (jt - lo) * 256:(jt - lo + 1) * 256],
                            lhsT=xtl8[:, 2 * u:2 * u + 2, jt * 128:(jt + 1) * 128],
                            rhs=wvt[:, 2 * u:2 * u + 2, :],
                            start=(u == 0), stop=(u == 1), perf_mode=DR,
                        )
                for j2 in range(lo, lo + 4):
                    nc.gpsimd.tensor_add(vts[j2 % 4][:, j2 // 4, :],
                                         vp[:, (j2 - lo) * 256:(j2 - lo + 1) * 256],
                                         bvt)

            # ---- S + exp phase: i-chunk outer, completely uniform.
            # Column sums for the first three i-chunks run as DVE reduces
            # over the just-written fp8 P tiles (saves the 187ns accumulator
            # drain on the pacing ScalarE); the last i-chunk keeps accum_out
            # so the per-j scale is ready the moment its exp retires.
            # P pair tiles hold (jt, jt+4) -- a 4-exp separation between a
            # pair tile's two writers, so a DVE reduce reading slice jm=0
            # never blocks the exp writing jm=1 even if it lags a few us.
            for c in range(NIC):
                for jt in range(JT):
                    sp = psum.tile([128, 2048], f32, tag="S", bufs=2,
                                   name="sp")
                    for s in range(4):
                        for u in range(2):
                            nc.tensor.matmul(
                                sp[:, s * 512:(s + 1) * 512],
                                lhsT=mts[:, 2 * u:2 * u + 2,
                                         jt * 128:(jt + 1) * 128],
                                rhs=xt8[:, 2 * u:2 * u + 2,
                                        c * 2048 + s * 512:c * 2048 + (s + 1) * 512],
                                start=(u == 0), stop=(u == 1), perf_mode=DR,
                            )
                    pslice = pts[jt % 4][:, jt // 4, c * 2048:(c + 1) * 2048]
                    col = colsq[:, jt * 4 + c:jt * 4 + c + 1]
                    if c < NIC - 1:
                        nc.scalar.activation(pslice, sp, AF.Exp,
                                             scale=EXPSCALE,
                                             bias=bqm[:, jt:jt + 1])
                        nc.vector.tensor_reduce(
                            col, pslice, mybir.AxisListType.X,
                            mybir.AluOpType.add)
                    else:
                        nc.scalar.activation(pslice, sp, AF.Exp,
                                             scale=EXPSCALE,
                                             bias=bqm[:, jt:jt + 1],
                                             accum_out=col)
                    if c == 0 and jt == 1:
                        v_mms(0)
                    if c == 0 and jt == 3:
                        v_mms(4)
                    if c == NIC - 1:
                        nc.vector.tensor_reduce(
                            csum[:, jt:jt + 1], colsq[:, jt * 4:(jt + 1) * 4],
                            mybir.AxisListType.X, mybir.AluOpType.add)
                        nc.vector.reciprocal(ginv[:, jt:jt + 1],
                                             csum[:, jt:jt + 1])
                        nc.vector.tensor_scalar_mul(
                            ginv[:, jt:jt + 1], ginv[:, jt:jt + 1], gsc)
                        if jt >= 4:
                            t = jt - 4
                            for jm in range(2):
                                nc.vector.tensor_scalar_mul(
                                    vss[t][:, jm, :], vts[t][:, jm, :],
                                    ginv[:, t + 4 * jm:t + 4 * jm + 1])

            # ---- PV phase: u''^T[c, i] partials, drained fp8 to pbuf ----
            drain_engs = [nc.gpsimd, nc.scalar, nc.vector]
            dri = [0]
            for g in range(4):
                stg = strm.tile([128, 2, 2, NL], f8, tag="stg", bufs=2,
                                name=f"stg{g}")  # [cp, r', ct, i']
                for ct in range(2):
                    pv = psum.tile([128, 2, NL], f32, tag="S", bufs=2,
                                   name="pv")    # [cp, r', i']
                    for sub in range(4):
                        ic = g * 4 + sub
                        for t in range(NPAIR):
                            nc.tensor.matmul(
                                pv[:, sub // 2, (sub % 2) * 512:(sub % 2 + 1) * 512],
                                lhsT=vss[t][:, :, ct * 128:(ct + 1) * 128],
                                rhs=pts[t][:, :, ic * 512:(ic + 1) * 512],
                                start=(t == 0), stop=(t == NPAIR - 1),
                                perf_mode=DR,
                            )
                    for rr in range(2):
                        e = drain_engs[dri[0] % 3]
                        dri[0] += 1
                        if e is nc.scalar:
                            nc.scalar.copy(stg[:, rr, ct, :], pv[:, rr, :])
                        else:
                            e.tensor_scalar_add(stg[:, rr, ct, :],
                                                pv[:, rr, :], 0.0)
                    gb = 2 * 2 * 128 * NL
                    nc.sync.dma_start(
                        out=pbuf[g * gb:(g + 1) * gb].rearrange(
                            "(r p c i) -> p r c i", r=2, p=128, c=2)[:, :, ct, :],
                        in_=stg[:, :, ct, :],
                    )

            # ---- single collective: sum partials, scatter rows to owners ----
            nc.gpsimd.collective_compute(
                "ReduceScatter",
                mybir.AluOpType.add,
                replica_groups=RG,
                ins=[pbuf[:]],
                outs=[rsout[:]],
            )

            # ---- final: y = x@(a W1W2) + u''@(8 W2)/(8 VS) + b'' ----
            # x-term + PE warm-up dummies run DURING the ReduceScatter
            fx = psum.tile([128, 2048], f32, tag="S", bufs=2, name="fx")
            for it in range(8):
                for u in range(4):
                    nc.tensor.matmul(
                        fx[:, it * 256:(it + 1) * 256],
                        lhsT=xtlb[:, u, it * 128:(it + 1) * 128],
                        rhs=w12t[:, u, :],
                        start=(u == 0), stop=(u == 3),
                    )
            for it in range(8):
                e = [nc.vector, nc.gpsimd][it % 2]
                e.tensor_add(xyt[:, it, :],
                             fx[:, it * 256:(it + 1) * 256], b2t)
            nc.sync.dma_start(
                out=uts,
                in_=rsout[:].rearrange("(p c i) -> p c i", p=128, c=2),
            )
            # keep the PE p-state at full clock across the collective (slow
            # f32 matmuls into a scratch psum tile, no data deps on the RS);
            # the last few shrink so the overshoot past uts-arrival is small
            fdum = psum.tile([128, 2048], f32, tag="S", bufs=2, name="fdum")
            for d in range(NDUMMY):
                nc.tensor.matmul(
                    fdum[:, 0:512],
                    lhsT=bpt[:, 0:128],
                    rhs=bpt[:, 0:512],
                    start=True, stop=True,
                )
            for d in range(8):
                nc.tensor.matmul(
                    fdum[:, 0:128],
                    lhsT=bpt[:, 0:128],
                    rhs=bpt[:, 0:128],
                    start=True, stop=True,
                )
            # one psum BANK per it-slice (512-f32 stride): the yst add that
            # reads slice it must not share a bank with the it+1 matmul or
            # the bank-granular WAR check serializes the projection stream
            fya = psum.tile([128, 2048], f32, tag="S", bufs=2, name="fya")
            fyb = psum.tile([128, 2048], f32, tag="S", bufs=2, name="fyb")
            # all 8 projection matmuls first (no add interleaved in between:
            # a psum-tile read by an add would stall later matmul writes),
            # then the adds stream on two engines, then the y stores
            def fsl(it):
                return [fya, fyb][it % 2][:, (it // 2) * 512:
                                          (it // 2) * 512 + 256]
            for it in range(8):
                nc.tensor.matmul(
                    fsl(it),
                    lhsT=uts[:, :, it * 128:(it + 1) * 128],
                    rhs=w2t[:, :, :],
                    start=True, stop=True, perf_mode=DR,
                )
            for b in range(4):
                yst = strm.tile([128, 2, C], f32, tag="yst", bufs=4,
                                name=f"yst{b}")
                for qq in range(2):
                    it = b * 2 + qq
                    e = [nc.vector, nc.gpsimd][it % 2]
                    e.scalar_tensor_tensor(
                        yst[:, qq, :],
                        fsl(it),
                        1.0 / (W2S * VS),
                        xyt[:, it, :],
                        op0=mybir.AluOpType.mult,
                        op1=mybir.AluOpType.add,
                    )
                nc.sync.dma_start(
                    out=y[b * 256:(b + 1) * 256, :].rearrange(
                        "(a p) c -> p a c", a=2, p=128),
                    in_=yst,
                )

    nc.finalize()
    return nc


def _get_nc():
    if "nc" not in _CACHED:
        _CACHED["nc"] = _build()
    return _CACHED["nc"]


def _prep_in_maps(x, W1, b1, Wq, bq, Wk, bk, Wv, bv, a, W2, b2):
    f32 = np.float32
    x = np.asarray(x, f32)
    W1 = np.asarray(W1, f32)
    b1 = np.asarray(b1, f32)
    av = f32(np.asarray(a, f32).reshape(-1)[0])

    def fold(Wx, bx):
        Wx, bx = np.asarray(Wx, f32), np.asarray(bx, f32)
        return W1 @ Wx, b1 @ Wx + bx

    Wqf, bqf = fold(Wq, bq)
    Wkf, bkf = fold(Wk, bk)
    Wvf, bvf = fold(Wv, bv)
    W2_ = np.asarray(W2, f32)
    W12 = av * (W1 @ W2_)
    b12 = av * (b1 @ W2_) + np.asarray(b2, f32)

    def pack(W, dtype, kt):
        return np.ascontiguousarray(
            W.reshape(kt, 128, W.shape[1]).transpose(1, 0, 2)).astype(dtype)

    def bcol(v2):
        return np.ascontiguousarray(v2.reshape(2, 128).T)

    # Gram-folded attention-score factors (see module docstring):
    #   S_psum = (256 Wq' Wk'^T x_l^T)^T-contracted-with-x, exp bias =
    #   x_l.(64 Wk' bq)/1024 + (bk.bq)/16 per local row j
    G = f32(256.0) * (Wqf @ Wkf.T)              # [512 hin, 512 din]
    gb = f32(256.0) * (Wqf @ bkf)               # [512 hin]
    wb = f32(64.0) * (Wkf @ bqf)                # [512 din]
    cb = f32(np.dot(bkf, bqf) / 16.0)           # scalar

    bp = np.zeros((128, NBP), f32)
    bp[:, 4] = (f32(1.0) - av) * f32(VS)
    bp[:, 5:5 + C] = np.broadcast_to(bvf, (128, C))
    bp[:, 5 + C:5 + 2 * C] = np.broadcast_to(b12, (128, C))
    bp[:, 5 + 2 * C:9 + 2 * C] = gb.reshape(4, 128).T
    bp[:, 9 + 2 * C] = cb

    xT = np.ascontiguousarray(x.T.reshape(4, 128, N).transpose(1, 0, 2))
    xT8 = xT.astype(FP8)

    shared = {
        "xT8": xT8,
        "g8": pack(np.ascontiguousarray(G.T), FP8, 4),
        "wb8": np.ascontiguousarray(
            wb.reshape(4, 128).T)[:, :, None].astype(FP8),
        "wv8": pack(Wvf, FP8, 4),
        "w12b": pack(W12, BF16, 4),
        "w2f": pack(W2_ * f32(W2S), FP8, 2),
        "bpk": bp,
    }
    maps = []
    for r in range(NCORES):
        sl = slice(r * NL, (r + 1) * NL)
        maps.append({
            **shared,
            "xTl8": np.ascontiguousarray(xT8[:, :, sl]),
            "xTlb": np.ascontiguousarray(xT[:, :, sl]).astype(BF16),
        })
    return maps


def kernel(**inputs) -> np.ndarray:
    from concourse.bass_utils import run_bass_kernel_spmd

    nc = _get_nc()
    in_maps = _prep_in_maps(**inputs)
    res = run_bass_kernel_spmd(nc, in_maps, list(range(NCORES)))
    return np.ascontiguousarray(
        np.concatenate(
            [res.results[r]["y"] for r in range(NCORES)], axis=0
        ).astype(np.float32)
    )


# revision 56
# speedup vs baseline: 1.0118x; 1.0118x over previous
"""Trainium2 Bass kernel for gated dense attention with dim=0 softmax.

Computation (reference):
    h = x @ W1 + b1
    q,k,v = h @ W{q,k,v} + b{q,k,v}
    w = (q @ k.T) / sqrt(256)
    attn = softmax(w, axis=0)          # normalizes over ROWS per column
    h2 = a*h + (1-a)*(attn @ v)
    out = h2 @ W2 + b2

Distribution strategy (chosen for the cost model's collective pricing:
every collective costs a flat ~15us + bytes/40GBps, AllReduce x1.875):

  Replicate x to all 8 cores as fp8 (4MB HBM->SBUF stream at 360GB/s is
  far cheaper than any AllGather at collective rates).  Core r owns
  COLUMN block J_r = [r*1024, (r+1)*1024) of the attention matrix: it
  computes q for ALL rows locally (x replicated), k,v only for its own
  rows, then S^r = k_r @ q_all^T -> [1024 j, 8192 i].  The dim=0
  softmax denominator sums over ALL i -- fully local in this layout (no
  stats AllReduce).  Each core forms the partial output
  u^r[i,:] = sum_{j in J_r} P[i,j] v'[j,:] for ALL i, and a single fp8
  ReduceScatter (256KB out, ~21.5us; reduction itself runs fp32) both
  sums the partials over cores and hands each core its own 1024 rows.

  Collectives: ONE ReduceScatter.  (Baseline: AG-k + AG-v + 4 stats
  AllReduces ~ 249us of serialized collective time.)

Schedule: the pacing engine is ScalarE doing the 8.4M exps/core.  exp
runs at [128,2048] grain from a 2-deep psum ping-pong, back-to-back at
1892ns; S matmuls (fp8 DoubleRow, ~1.8us/chunk) hide underneath, and
q-chunk production is interleaved INTO the S stream so the first exp
fires ~11us in.  Column sums run on the otherwise-idle Pool engine
(tensor_reduce over the just-written fp8 P tiles) except the last
i-chunk, which uses the exp's accum_out so the per-j scale is ready
the moment its final exp retires.  During the ReduceScatter the PE
computes the residual x@(a*W1W2) term and then chews dummy f32 matmuls
to stay at full clock for the post-collective projection.

Weight folding (host, O(weights) only): q = x @ (W1@Wq) + (b1@Wq + bq),
same for k,v; y = x @ (a*W1@W2) + u'' @ (8*W2) / (8*VS) + (a*b1@W2+b2)
with u'' = VS*(1-a)*u carried through the fp8 ReduceScatter.
"""

import numpy as np
import ml_dtypes

BF16 = ml_dtypes.bfloat16
FP8 = ml_dtypes.float8_e4m3fn

N, D, H, C = 8192, 512, 256, 256
NCORES = 8
NL = N // NCORES          # 1024 rows/columns per core
JT = NL // 128            # 8 local j-tiles
NPAIR = JT // 2           # 4 DoubleRow j-pairs
NIC = N // 2048           # 4 i-chunks at the 2048-wide exp grain
WS = 16.0                 # fp8 range scale on folded Wq/Wk
VS = 4096.0               # range shim on v' and u'' (u'' stays in fp8 range)
W2S = 8.0                 # fp8 range scale on W2
EXPSCALE = 1.0 / (WS * WS * 16.0)   # restores exp(q.k/sqrt(256))
NBP = 10 + 2 * C          # packed bias columns (gsc, bv, b12, gb x4, cb)
NDUMMY = 21               # f32 warm-up matmuls spanning the ReduceScatter
NDSMALL = 12              # short trailing warm-ups (fine-grained landing)

_CACHED = {}


def _build():
    import concourse.mybir as mybir
    from concourse import bacc
    from concourse.tile import TileContext

    dt = mybir.dt
    AF = mybir.ActivationFunctionType
    DR = mybir.MatmulPerfMode.DoubleRow
    f32, bf, f8 = dt.float32, dt.bfloat16, dt.float8e4
    RG = [list(range(NCORES))]

    nc = bacc.Bacc(None, target_bir_lowering=False, num_devices=NCORES)

    # ---------------- I/O (per core) ----------------
    xT8 = nc.declare_dram_parameter("xT8", [128, 4, N], f8, isOutput=False)
    xTl8 = nc.declare_dram_parameter("xTl8", [128, 4, NL], f8, isOutput=False)
    xTlb = nc.declare_dram_parameter("xTlb", [128, 4, NL], bf, isOutput=False)
    g8 = nc.declare_dram_parameter("g8", [128, 4, D], f8, isOutput=False)
    wb8 = nc.declare_dram_parameter("wb8", [128, 4, 1], f8, isOutput=False)
    wv8 = nc.declare_dram_parameter("wv8", [128, 4, H], f8, isOutput=False)
    w12b = nc.declare_dram_parameter("w12b", [128, 4, C], bf, isOutput=False)
    w2f = nc.declare_dram_parameter("w2f", [128, 2, C], f8, isOutput=False)
    bpk = nc.declare_dram_parameter("bpk", [128, NBP], f32, isOutput=False)
    y = nc.declare_dram_parameter("y", [NL, C], f32, isOutput=True)

    # partial outputs, laid out [rank][cp, ct, i'] so the ReduceScatter
    # shard for rank r is u''^T for its own rows, lhsT-ready
    pbuf = nc.dram_tensor("pbuf", [NCORES * 2 * 128 * NL], f8)
    rsout = nc.dram_tensor("rsout", [2 * 128 * NL], f8)

    with TileContext(nc) as tc:
        with (
            tc.tile_pool(name="cst", bufs=1) as cst,
            tc.tile_pool(name="big", bufs=1) as big,
            tc.tile_pool(name="strm", bufs=1) as strm,
            tc.tile_pool(name="psum", bufs=1, space="PSUM") as psum,
        ):
            # ---- SBUF residents ----
            bpt = cst.tile([128, NBP], f32, tag="bpt", name="bpt")
            g8t = cst.tile([128, 4, D], f8, tag="g8t", name="g8t")
            wb8t = cst.tile([128, 4, 1], f8, tag="wb8t", name="wb8t")
            wvt = cst.tile([128, 4, H], f8, tag="wvt", name="wvt")
            w12t = cst.tile([128, 4, C], bf, tag="w12t", name="w12t")
            w2t = cst.tile([128, 2, C], f8, tag="w2t", name="w2t")
            scr = cst.tile([128, 1], f32, tag="scr", name="scr")

            xt8 = big.tile([128, 4, N], f8, tag="xt8", name="xt8")
            xtl8 = big.tile([128, 4, NL], f8, tag="xtl8", name="xtl8")
            xtlb = big.tile([128, 4, NL], bf, tag="xtlb", name="xtlb")
            mts = big.tile([128, 4, NL], f8, tag="mts", name="mts")
            bqm = big.tile([128, JT], f32, tag="bqm", name="bqm")
            vts = [big.tile([128, 2, C], f8, tag=f"vts{t}", name=f"vts{t}")
                   for t in range(NPAIR)]
            vss = [big.tile([128, 2, C], f8, tag=f"vss{t}", name=f"vss{t}")
                   for t in range(NPAIR)]
            pts = [big.tile([128, 2, N], f8, tag=f"pts{t}", name=f"pts{t}")
                   for t in range(NPAIR)]
            colsq = big.tile([128, 4 * JT], f32, tag="colsq", name="colsq")
            csum = big.tile([128, JT], f32, tag="csum", name="csum")
            ginv = big.tile([128, JT], f32, tag="ginv", name="ginv")
            uts = big.tile([128, 2, NL], f8, tag="uts", name="uts")
            xyt = big.tile([128, 8, C], f32, tag="xyt", name="xyt")

            gsc = bpt[:, 4:5]
            bvt = bpt[:, 5:5 + C]
            b2t = bpt[:, 5 + C:5 + 2 * C]
            gbc = lambda t: bpt[:, 5 + 2 * C + t:6 + 2 * C + t]
            cbc = bpt[:, 9 + 2 * C:10 + 2 * C]

            # ---- ACT table preload: Exp table load off the critical path
            nc.vector.memset(scr, 0.0)
            nc.scalar.activation(scr, scr, AF.Exp, scale=0.0)

            # ---- input DMAs ----
            nc.scalar.dma_start(out=g8t, in_=g8[:])
            nc.scalar.dma_start(out=wb8t, in_=wb8[:])
            nc.scalar.dma_start(out=bpt, in_=bpk[:])
            nc.gpsimd.dma_start(out=xtl8, in_=xTl8[:])
            nc.gpsimd.dma_start(out=wvt, in_=wv8[:])
            for c in range(NIC):
                sl = slice(c * 2048, (c + 1) * 2048)
                nc.sync.dma_start(out=xt8[:, :, sl], in_=xT8[:, :, sl])
            nc.sync.dma_start(out=xtlb, in_=xTlb[:])
            nc.sync.dma_start(out=w12t, in_=w12b[:])
            nc.sync.dma_start(out=w2t, in_=w2f[:])

            # ---- head: M^T = G @ x_l^T + gb where G = 256*(Wq' Wk'^T) is
            # host-folded, so S contracts M directly against the replicated
            # x (neither q nor k ever materializes; the q-bias row enters
            # via the exp's per-partition bias, itself a folded matvec).
            # exp-bias matvec first: bias_j = x_l[j].(Wk' bq)/16 + bk.bq/16
            bqp = psum.tile([128, 2048], f32, tag="S", bufs=2, name="bqp")
            for jt in range(JT):
                for u in range(2):
                    nc.tensor.matmul(
                        bqp[:, jt:jt + 1],
                        lhsT=xtl8[:, 2 * u:2 * u + 2, jt * 128:(jt + 1) * 128],
                        rhs=wb8t[:, 2 * u:2 * u + 2, :],
                        start=(u == 0), stop=(u == 1), perf_mode=DR,
                    )
            mp = [psum.tile([128, 2048], f32, tag="S", bufs=2, name=f"mp{h}")
                  for h in range(2)]
            for hk in range(4):
                for jc in range(2):
                    for u in range(2):
                        nc.tensor.matmul(
                            mp[hk // 2][:, (hk % 2) * 1024 + jc * 512:
                                        (hk % 2) * 1024 + (jc + 1) * 512],
                            lhsT=g8t[:, 2 * u:2 * u + 2, hk * 128:(hk + 1) * 128],
                            rhs=xtl8[:, 2 * u:2 * u + 2, jc * 512:(jc + 1) * 512],
                            start=(u == 0), stop=(u == 1), perf_mode=DR,
                        )
            nc.vector.tensor_scalar(
                bqm, bqp[:, 0:JT], 1.0 / 1024.0, cbc,
                op0=mybir.AluOpType.mult, op1=mybir.AluOpType.add)
            # three-way split so the last M quarter lands ~1us earlier
            for h in range(2):
                for e2 in range(2):
                    hq = 2 * h + e2
                    src = mp[h][:, e2 * 1024:(e2 + 1) * 1024]
                    if hq == 2:
                        nc.scalar.activation(mts[:, hq, :], src, AF.Identity,
                                             bias=gbc(hq))
                    elif hq == 0:
                        nc.vector.tensor_scalar_add(mts[:, hq, :], src,
                                                    gbc(hq))
                    else:
                        nc.gpsimd.tensor_scalar_add(mts[:, hq, :], src,
                                                    gbc(hq))

            def v_mms(lo):
                # v[j_local, c] in halves slotted under the exp pipeline's
                # PE slack (a full 16-matmul burst would starve one exp)
                vp = psum.tile([128, 2048], f32, tag="S", bufs=2, name="vp")
                for jt in range(lo, lo + 4):
                    for u in range(2):
                        nc.tensor.matmul(
                            vp[:, (jt - lo) * 256:(jt - lo + 1) * 256],
                            lhsT=xtl8[:, 2 * u:2 * u + 2, jt * 128:(jt + 1) * 128],
                            rhs=wvt[:, 2 * u:2 * u + 2, :],
                            start=(u == 0), stop=(u == 1), perf_mode=DR,
                        )
                for j2 in range(lo, lo + 4):
                    nc.gpsimd.tensor_add(vts[j2 % 4][:, j2 // 4, :],
                                         vp[:, (j2 - lo) * 256:(j2 - lo + 1) * 256],
                                         bvt)

            # ---- S + exp phase: i-chunk outer, completely uniform.
            # Column sums for the first three i-chunks run as DVE reduces
            # over the just-written fp8 P tiles (saves the 187ns accumulator
            # drain on the pacing ScalarE); the last i-chunk keeps accum_out
            # so the per-j scale is ready the moment its exp retires.
            # P pair tiles hold (jt, jt+4) -- a 4-exp separation between a
            # pair tile's two writers, so a DVE reduce reading slice jm=0
            # never blocks the exp writing jm=1 even if it lags a few us.
            for c in range(NIC):
                for jt in range(JT):
                    sp = psum.tile([128, 2048], f32, tag="S", bufs=2,
                                   name="sp")
                    for s in range(4):
                        for u in range(2):
                            nc.tensor.matmul(
                                sp[:, s * 512:(s + 1) * 512],
                                lhsT=mts[:, 2 * u:2 * u + 2,
                                         jt * 128:(jt + 1) * 128],
                                rhs=xt8[:, 2 * u:2 * u + 2,
                                        c * 2048 + s * 512:c * 2048 + (s + 1) * 512],
                                start=(u == 0), stop=(u == 1), perf_mode=DR,
                            )
                    pslice = pts[jt % 4][:, jt // 4, c * 2048:(c + 1) * 2048]
                    col = colsq[:, jt * 4 + c:jt * 4 + c + 1]
                    if c < NIC - 1:
                        nc.scalar.activation(pslice, sp, AF.Exp,
                                             scale=EXPSCALE,
                                             bias=bqm[:, jt:jt + 1])
                        nc.vector.tensor_reduce(
                            col, pslice, mybir.AxisListType.X,
                            mybir.AluOpType.add)
                    else:
                        nc.scalar.activation(pslice, sp, AF.Exp,
                                             scale=EXPSCALE,
                                             bias=bqm[:, jt:jt + 1],
                                             accum_out=col)
                    # v production rides the roomier 2079ns accum-exp windows
                    # of the last i-chunk; vts is complete well before the
                    # first vss scaling at jt==4
                    if c == NIC - 1 and jt == 0:
                        v_mms(0)
                    if c == NIC - 1 and jt == 1:
                        v_mms(4)
                    if c == NIC - 1:
                        nc.vector.tensor_reduce(
                            csum[:, jt:jt + 1], colsq[:, jt * 4:(jt + 1) * 4],
                            mybir.AxisListType.X, mybir.AluOpType.add)
                        nc.vector.reciprocal(ginv[:, jt:jt + 1],
                                             csum[:, jt:jt + 1])
                        nc.vector.tensor_scalar_mul(
                            ginv[:, jt:jt + 1], ginv[:, jt:jt + 1], gsc)
                        if jt >= 4:
                            t = jt - 4
                            for jm in range(2):
                                nc.vector.tensor_scalar_mul(
                                    vss[t][:, jm, :], vts[t][:, jm, :],
                                    ginv[:, t + 4 * jm:t + 4 * jm + 1])

            # ---- PV phase: u''^T[c, i] partials, drained fp8 to pbuf ----
            drain_engs = [nc.gpsimd, nc.scalar, nc.vector]
            dri = [0]
            for g in range(4):
                stg = strm.tile([128, 2, 2, NL], f8, tag="stg", bufs=2,
                                name=f"stg{g}")  # [cp, r', ct, i']
                for ct in range(2):
                    pv = psum.tile([128, 2, NL], f32, tag="S", bufs=2,
                                   name="pv")    # [cp, r', i']
                    for sub in range(4):
                        ic = g * 4 + sub
                        for t in range(NPAIR):
                            nc.tensor.matmul(
                                pv[:, sub // 2, (sub % 2) * 512:(sub % 2 + 1) * 512],
                                lhsT=vss[t][:, :, ct * 128:(ct + 1) * 128],
                                rhs=pts[t][:, :, ic * 512:(ic + 1) * 512],
                                start=(t == 0), stop=(t == NPAIR - 1),
                                perf_mode=DR,
                            )
                    for rr in range(2):
                        e = drain_engs[dri[0] % 3]
                        dri[0] += 1
                        if e is nc.scalar:
                            nc.scalar.copy(stg[:, rr, ct, :], pv[:, rr, :])
                        else:
                            e.tensor_scalar_add(stg[:, rr, ct, :],
                                                pv[:, rr, :], 0.0)
                    gb = 2 * 2 * 128 * NL
                    nc.sync.dma_start(
                        out=pbuf[g * gb:(g + 1) * gb].rearrange(
                            "(r p c i) -> p r c i", r=2, p=128, c=2)[:, :, ct, :],
                        in_=stg[:, :, ct, :],
                    )

            # ---- single collective: sum partials, scatter rows to owners ----
            nc.gpsimd.collective_compute(
                "ReduceScatter",
                mybir.AluOpType.add,
                replica_groups=RG,
                ins=[pbuf[:]],
                outs=[rsout[:]],
            )

            # ---- final: y = x@(a W1W2) + u''@(8 W2)/(8 VS) + b'' ----
            # x-term + PE warm-up dummies run DURING the ReduceScatter
            fx = psum.tile([128, 2048], f32, tag="S", bufs=2, name="fx")
            for it in range(8):
                for u in range(4):
                    nc.tensor.matmul(
                        fx[:, it * 256:(it + 1) * 256],
                        lhsT=xtlb[:, u, it * 128:(it + 1) * 128],
                        rhs=w12t[:, u, :],
                        start=(u == 0), stop=(u == 3),
                    )
            for it in range(8):
                e = [nc.vector, nc.gpsimd][it % 2]
                e.tensor_add(xyt[:, it, :],
                             fx[:, it * 256:(it + 1) * 256], b2t)
            nc.sync.dma_start(
                out=uts,
                in_=rsout[:].rearrange("(p c i) -> p c i", p=128, c=2),
            )
            # keep the PE p-state at full clock across the collective (slow
            # f32 matmuls into a scratch psum tile, no data deps on the RS);
            # the last few shrink so the overshoot past uts-arrival is small
            fdum = psum.tile([128, 2048], f32, tag="S", bufs=2, name="fdum")
            for d in range(NDUMMY):
                nc.tensor.matmul(
                    fdum[:, 0:512],
                    lhsT=bpt[:, 0:128],
                    rhs=bpt[:, 0:512],
                    start=True, stop=True,
                )
            for d in range(NDSMALL):
                nc.tensor.matmul(
                    fdum[:, 0:128],
                    lhsT=bpt[:, 0:128],
                    rhs=bpt[:, 0:128],
                    start=True, stop=True,
                )
            # one psum BANK per it-slice (512-f32 stride): the yst add that
            # reads slice it must not share a bank with the it+1 matmul or
            # the bank-granular WAR check serializes the projection stream
            fya = psum.tile([128, 2048], f32, tag="S", bufs=2, name="fya")
            fyb = psum.tile([128, 2048], f32, tag="S", bufs=2, name="fyb")
            # all 8 projection matmuls first (no add interleaved in between:
            # a psum-tile read by an add would stall later matmul writes),
            # then the adds stream on two engines, then the y stores
            def fsl(it):
                return [fya, fyb][it % 2][:, (it // 2) * 512:
                                          (it // 2) * 512 + 256]
            for it in range(8):
                nc.tensor.matmul(
                    fsl(it),
                    lhsT=uts[:, :, it * 128:(it + 1) * 128],
                    rhs=w2t[:, :, :],
                    start=True, stop=True, perf_mode=DR,
                )
            for b in range(4):
                yst = strm.tile([128, 2, C], f32, tag="yst", bufs=4,
                                name=f"yst{b}")
                for qq in range(2):
                    it = b * 2 + qq
                    e = [nc.vector, nc.gpsimd][it % 2]
                    e.scalar_tensor_tensor(
                        yst[:, qq, :],
                        fsl(it),
                        1.0 / (W2S * VS),
                        xyt[:, it, :],
                        op0=mybir.AluOpType.mult,
                        op1=mybir.AluOpType.add,
                    )
                nc.sync.dma_start(
                    out=y[b * 256:(b + 1) * 256, :].rearrange(
                        "(a p) c -> p a c", a=2, p=128),
                    in_=yst,
                )

    nc.finalize()
    return nc


def _get_nc():
    if "nc" not in _CACHED:
        _CACHED["nc"] = _build()
    return _CACHED["nc"]


def _prep_in_maps(x, W1, b1, Wq, bq, Wk, bk, Wv, bv, a, W2, b2):
    f32 = np.float32
    x = np.asarray(x, f32)
    W1 = np.asarray(W1, f32)
    b1 = np.asarray(b1, f32)
    av = f32(np.asarray(a, f32).reshape(-1)[0])

    def fold(Wx, bx):
        Wx, bx = np.asarray(Wx, f32), np.asarray(bx, f32)
        return W1 @ Wx, b1 @ Wx + bx

    Wqf, bqf = fold(Wq, bq)
    Wkf, bkf = fold(Wk, bk)
    Wvf, bvf = fold(Wv, bv)
    W2_ = np.asarray(W2, f32)
    W12 = av * (W1 @ W2_)
    b12 = av * (b1 @ W2_) + np.asarray(b2, f32)

    def pack(W, dtype, kt):
        return np.ascontiguousarray(
            W.reshape(kt, 128, W.shape[1]).transpose(1, 0, 2)).astype(dtype)

    def bcol(v2):
        return np.ascontiguousarray(v2.reshape(2, 128).T)

    # Gram-folded attention-score factors (see module docstring):
    #   S_psum = (256 Wq' Wk'^T x_l^T)^T-contracted-with-x, exp bias =
    #   x_l.(64 Wk' bq)/1024 + (bk.bq)/16 per local row j
    G = f32(256.0) * (Wqf @ Wkf.T)              # [512 hin, 512 din]
    gb = f32(256.0) * (Wqf @ bkf)               # [512 hin]
    wb = f32(64.0) * (Wkf @ bqf)                # [512 din]
    cb = f32(np.dot(bkf, bqf) / 16.0)           # scalar

    bp = np.zeros((128, NBP), f32)
    bp[:, 4] = (f32(1.0) - av) * f32(VS)
    bp[:, 5:5 + C] = np.broadcast_to(bvf, (128, C))
    bp[:, 5 + C:5 + 2 * C] = np.broadcast_to(b12, (128, C))
    bp[:, 5 + 2 * C:9 + 2 * C] = gb.reshape(4, 128).T
    bp[:, 9 + 2 * C] = cb

    xT = np.ascontiguousarray(x.T.reshape(4, 128, N).transpose(1, 0, 2))
    xT8 = xT.astype(FP8)

    shared = {
        "xT8": xT8,
        "g8": pack(np.ascontiguousarray(G.T), FP8, 4),
        "wb8": np.ascontiguousarray(
            wb.reshape(4, 128).T)[:, :, None].astype(FP8),
        "wv8": pack(Wvf, FP8, 4),
        "w12b": pack(W12, BF16, 4),
        "w2f": pack(W2_ * f32(W2S), FP8, 2),
        "bpk": bp,
    }
    maps = []
    for r in range(NCORES):
        sl = slice(r * NL, (r + 1) * NL)
        maps.append({
            **shared,
            "xTl8": np.ascontiguousarray(xT8[:, :, sl]),
            "xTlb": np.ascontiguousarray(xT[:, :, sl]).astype(BF16),
        })
    return maps


def kernel(**inputs) -> np.ndarray:
    from concourse.bass_utils import run_bass_kernel_spmd

    nc = _get_nc()
    in_maps = _prep_in_maps(**inputs)
    res = run_bass_kernel_spmd(nc, in_maps, list(range(NCORES)))
    return np.ascontiguousarray(
        np.concatenate(
            [res.results[r]["y"] for r in range(NCORES)], axis=0
        ).astype(np.float32)
    )


# revision 58
# speedup vs baseline: 1.0201x; 1.0082x over previous
"""Trainium2 Bass kernel for gated dense attention with dim=0 softmax.

Computation (reference):
    h = x @ W1 + b1
    q,k,v = h @ W{q,k,v} + b{q,k,v}
    w = (q @ k.T) / sqrt(256)
    attn = softmax(w, axis=0)          # normalizes over ROWS per column
    h2 = a*h + (1-a)*(attn @ v)
    out = h2 @ W2 + b2

Distribution strategy (chosen for the cost model's collective pricing:
every collective costs a flat ~15us + bytes/40GBps, AllReduce x1.875):

  Replicate x to all 8 cores as fp8 (4MB HBM->SBUF stream at 360GB/s is
  far cheaper than any AllGather at collective rates).  Core r owns
  COLUMN block J_r = [r*1024, (r+1)*1024) of the attention matrix: it
  computes q for ALL rows locally (x replicated), k,v only for its own
  rows, then S^r = k_r @ q_all^T -> [1024 j, 8192 i].  The dim=0
  softmax denominator sums over ALL i -- fully local in this layout (no
  stats AllReduce).  Each core forms the partial output
  u^r[i,:] = sum_{j in J_r} P[i,j] v'[j,:] for ALL i, and a single fp8
  ReduceScatter (256KB out, ~21.5us; reduction itself runs fp32) both
  sums the partials over cores and hands each core its own 1024 rows.

  Collectives: ONE ReduceScatter.  (Baseline: AG-k + AG-v + 4 stats
  AllReduces ~ 249us of serialized collective time.)

Schedule: the pacing engine is ScalarE doing the 8.4M exps/core.  exp
runs at [128,2048] grain from a 2-deep psum ping-pong, back-to-back at
1892ns; S matmuls (fp8 DoubleRow, ~1.8us/chunk) hide underneath, and
q-chunk production is interleaved INTO the S stream so the first exp
fires ~11us in.  Column sums run on the otherwise-idle Pool engine
(tensor_reduce over the just-written fp8 P tiles) except the last
i-chunk, which uses the exp's accum_out so the per-j scale is ready
the moment its final exp retires.  During the ReduceScatter the PE
computes the residual x@(a*W1W2) term and then chews dummy f32 matmuls
to stay at full clock for the post-collective projection.

Weight folding (host, O(weights) only): q = x @ (W1@Wq) + (b1@Wq + bq),
same for k,v; y = x @ (a*W1@W2) + u'' @ (8*W2) / (8*VS) + (a*b1@W2+b2)
with u'' = VS*(1-a)*u carried through the fp8 ReduceScatter.
"""

import numpy as np
import ml_dtypes

BF16 = ml_dtypes.bfloat16
FP8 = ml_dtypes.float8_e4m3fn

N, D, H, C = 8192, 512, 256, 256
NCORES = 8
NL = N // NCORES          # 1024 rows/columns per core
JT = NL // 128            # 8 local j-tiles
NPAIR = JT // 2           # 4 DoubleRow j-pairs
NIC = N // 2048           # 4 i-chunks at the 2048-wide exp grain
WS = 16.0                 # fp8 range scale on folded Wq/Wk
VS = 4096.0               # range shim on v' and u'' (u'' stays in fp8 range)
W2S = 8.0                 # fp8 range scale on W2
EXPSCALE = 1.0 / (WS * WS * 16.0)   # restores exp(q.k/sqrt(256))
NBP = 10 + 2 * C          # packed bias columns (gsc, bv, b12, gb x4, cb)
NDUMMY = 26               # f32 warm-up matmuls spanning the ReduceScatter
NDSMALL = 12              # short trailing warm-ups (fine-grained landing)

_CACHED = {}


def _build():
    import concourse.mybir as mybir
    from concourse import bacc
    from concourse.tile import TileContext

    dt = mybir.dt
    AF = mybir.ActivationFunctionType
    DR = mybir.MatmulPerfMode.DoubleRow
    f32, bf, f8 = dt.float32, dt.bfloat16, dt.float8e4
    RG = [list(range(NCORES))]

    nc = bacc.Bacc(None, target_bir_lowering=False, num_devices=NCORES)

    # ---------------- I/O (per core) ----------------
    xT8 = nc.declare_dram_parameter("xT8", [128, 4, N], f8, isOutput=False)
    xTl8 = nc.declare_dram_parameter("xTl8", [128, 4, NL], f8, isOutput=False)
    xTlb = nc.declare_dram_parameter("xTlb", [128, 4, NL], bf, isOutput=False)
    g8 = nc.declare_dram_parameter("g8", [128, 4, D], f8, isOutput=False)
    wb8 = nc.declare_dram_parameter("wb8", [128, 4, 1], f8, isOutput=False)
    wv8 = nc.declare_dram_parameter("wv8", [128, 4, H], f8, isOutput=False)
    w12b = nc.declare_dram_parameter("w12b", [128, 4, C], bf, isOutput=False)
    w2f = nc.declare_dram_parameter("w2f", [128, 2, C], f8, isOutput=False)
    bpk = nc.declare_dram_parameter("bpk", [128, NBP], f32, isOutput=False)
    y = nc.declare_dram_parameter("y", [NL, C], f32, isOutput=True)

    # partial outputs, laid out [rank][cp, ct, i'] so the ReduceScatter
    # shard for rank r is u''^T for its own rows, lhsT-ready
    pbuf = nc.dram_tensor("pbuf", [NCORES * 2 * 128 * NL], f8)
    rsout = nc.dram_tensor("rsout", [2 * 128 * NL], f8)

    with TileContext(nc) as tc:
        with (
            tc.tile_pool(name="cst", bufs=1) as cst,
            tc.tile_pool(name="big", bufs=1) as big,
            tc.tile_pool(name="strm", bufs=1) as strm,
            tc.tile_pool(name="psum", bufs=1, space="PSUM") as psum,
        ):
            # ---- SBUF residents ----
            bpt = cst.tile([128, NBP], f32, tag="bpt", name="bpt")
            g8t = cst.tile([128, 4, D], f8, tag="g8t", name="g8t")
            wb8t = cst.tile([128, 4, 1], f8, tag="wb8t", name="wb8t")
            wvt = cst.tile([128, 4, H], f8, tag="wvt", name="wvt")
            w12t = cst.tile([128, 4, C], bf, tag="w12t", name="w12t")
            w2t = cst.tile([128, 2, C], f8, tag="w2t", name="w2t")
            scr = cst.tile([128, 1], f32, tag="scr", name="scr")

            xt8 = big.tile([128, 4, N], f8, tag="xt8", name="xt8")
            xtl8 = big.tile([128, 4, NL], f8, tag="xtl8", name="xtl8")
            xtlb = big.tile([128, 4, NL], bf, tag="xtlb", name="xtlb")
            mts = big.tile([128, 4, NL], f8, tag="mts", name="mts")
            bqm = big.tile([128, JT], f32, tag="bqm", name="bqm")
            vts = [big.tile([128, 2, C], f8, tag=f"vts{t}", name=f"vts{t}")
                   for t in range(NPAIR)]
            vss = [big.tile([128, 2, C], f8, tag=f"vss{t}", name=f"vss{t}")
                   for t in range(NPAIR)]
            pts = [big.tile([128, 2, N], f8, tag=f"pts{t}", name=f"pts{t}")
                   for t in range(NPAIR)]
            colsq = big.tile([128, 4 * JT], f32, tag="colsq", name="colsq")
            csum = big.tile([128, JT], f32, tag="csum", name="csum")
            ginv = big.tile([128, JT], f32, tag="ginv", name="ginv")
            uts = big.tile([128, 2, NL], f8, tag="uts", name="uts")
            xyt = big.tile([128, 8, C], f32, tag="xyt", name="xyt")

            gsc = bpt[:, 4:5]
            bvt = bpt[:, 5:5 + C]
            b2t = bpt[:, 5 + C:5 + 2 * C]
            gbc = lambda t: bpt[:, 5 + 2 * C + t:6 + 2 * C + t]
            cbc = bpt[:, 9 + 2 * C:10 + 2 * C]

            # ---- ACT table preload: Exp table load off the critical path
            nc.vector.memset(scr, 0.0)
            nc.scalar.activation(scr, scr, AF.Exp, scale=0.0)

            # ---- input DMAs ----
            nc.scalar.dma_start(out=g8t, in_=g8[:])
            nc.scalar.dma_start(out=wb8t, in_=wb8[:])
            nc.scalar.dma_start(out=bpt, in_=bpk[:])
            nc.gpsimd.dma_start(out=xtl8, in_=xTl8[:])
            nc.gpsimd.dma_start(out=wvt, in_=wv8[:])
            for c in range(NIC):
                sl = slice(c * 2048, (c + 1) * 2048)
                nc.sync.dma_start(out=xt8[:, :, sl], in_=xT8[:, :, sl])
            nc.sync.dma_start(out=xtlb, in_=xTlb[:])
            nc.sync.dma_start(out=w12t, in_=w12b[:])
            nc.sync.dma_start(out=w2t, in_=w2f[:])

            # ---- head: M^T = G @ x_l^T + gb where G = 256*(Wq' Wk'^T) is
            # host-folded, so S contracts M directly against the replicated
            # x (neither q nor k ever materializes; the q-bias row enters
            # via the exp's per-partition bias, itself a folded matvec).
            # exp-bias matvec first: bias_j = x_l[j].(Wk' bq)/16 + bk.bq/16
            bqp = psum.tile([128, 2048], f32, tag="S", bufs=2, name="bqp")
            for jt in range(JT):
                for u in range(2):
                    nc.tensor.matmul(
                        bqp[:, jt:jt + 1],
                        lhsT=xtl8[:, 2 * u:2 * u + 2, jt * 128:(jt + 1) * 128],
                        rhs=wb8t[:, 2 * u:2 * u + 2, :],
                        start=(u == 0), stop=(u == 1), perf_mode=DR,
                    )
            mp = [psum.tile([128, 2048], f32, tag="S", bufs=2, name=f"mp{h}")
                  for h in range(2)]
            for hk in range(4):
                for jc in range(2):
                    for u in range(2):
                        nc.tensor.matmul(
                            mp[hk // 2][:, (hk % 2) * 1024 + jc * 512:
                                        (hk % 2) * 1024 + (jc + 1) * 512],
                            lhsT=g8t[:, 2 * u:2 * u + 2, hk * 128:(hk + 1) * 128],
                            rhs=xtl8[:, 2 * u:2 * u + 2, jc * 512:(jc + 1) * 512],
                            start=(u == 0), stop=(u == 1), perf_mode=DR,
                        )
            nc.vector.tensor_scalar(
                bqm, bqp[:, 0:JT], 1.0 / 1024.0, cbc,
                op0=mybir.AluOpType.mult, op1=mybir.AluOpType.add)
            # keep the PE busy while the M converts drain so the first S
            # matmuls run at full clock (writes a scratch bank of bqp)
            for d in range(6):
                nc.tensor.matmul(
                    bqp[:, 512:1024],
                    lhsT=g8t[:, 0:2, 0:128],
                    rhs=xtl8[:, 0:2, 0:512],
                    start=True, stop=True, perf_mode=DR,
                )
            # three-way split so the last M quarter lands ~1us earlier
            for h in range(2):
                for e2 in range(2):
                    hq = 2 * h + e2
                    src = mp[h][:, e2 * 1024:(e2 + 1) * 1024]
                    if hq == 2:
                        nc.scalar.activation(mts[:, hq, :], src, AF.Identity,
                                             bias=gbc(hq))
                    elif hq == 0:
                        nc.vector.tensor_scalar_add(mts[:, hq, :], src,
                                                    gbc(hq))
                    else:
                        nc.gpsimd.tensor_scalar_add(mts[:, hq, :], src,
                                                    gbc(hq))

            def v_mms(lo):
                # v[j_local, c] in halves slotted under the exp pipeline's
                # PE slack (a full 16-matmul burst would starve one exp)
                vp = psum.tile([128, 2048], f32, tag="S", bufs=2, name="vp")
                for jt in range(lo, lo + 4):
                    for u in range(2):
                        nc.tensor.matmul(
                            vp[:, (jt - lo) * 256:(jt - lo + 1) * 256],
                            lhsT=xtl8[:, 2 * u:2 * u + 2, jt * 128:(jt + 1) * 128],
                            rhs=wvt[:, 2 * u:2 * u + 2, :],
                            start=(u == 0), stop=(u == 1), perf_mode=DR,
                        )
                for j2 in range(lo, lo + 4):
                    nc.gpsimd.tensor_add(vts[j2 % 4][:, j2 // 4, :],
                                         vp[:, (j2 - lo) * 256:(j2 - lo + 1) * 256],
                                         bvt)

            # ---- S + exp phase: i-chunk outer, completely uniform.
            # Column sums for the first three i-chunks run as DVE reduces
            # over the just-written fp8 P tiles (saves the 187ns accumulator
            # drain on the pacing ScalarE); the last i-chunk keeps accum_out
            # so the per-j scale is ready the moment its exp retires.
            # P pair tiles hold (jt, jt+4) -- a 4-exp separation between a
            # pair tile's two writers, so a DVE reduce reading slice jm=0
            # never blocks the exp writing jm=1 even if it lags a few us.
            for c in range(NIC):
                for jt in range(JT):
                    sp = psum.tile([128, 2048], f32, tag="S", bufs=2,
                                   name="sp")
                    for s in range(4):
                        for u in range(2):
                            nc.tensor.matmul(
                                sp[:, s * 512:(s + 1) * 512],
                                lhsT=mts[:, 2 * u:2 * u + 2,
                                         jt * 128:(jt + 1) * 128],
                                rhs=xt8[:, 2 * u:2 * u + 2,
                                        c * 2048 + s * 512:c * 2048 + (s + 1) * 512],
                                start=(u == 0), stop=(u == 1), perf_mode=DR,
                            )
                    pslice = pts[jt % 4][:, jt // 4, c * 2048:(c + 1) * 2048]
                    col = colsq[:, jt * 4 + c:jt * 4 + c + 1]
                    if c < NIC - 1:
                        nc.scalar.activation(pslice, sp, AF.Exp,
                                             scale=EXPSCALE,
                                             bias=bqm[:, jt:jt + 1])
                        nc.vector.tensor_reduce(
                            col, pslice, mybir.AxisListType.X,
                            mybir.AluOpType.add)
                    else:
                        nc.scalar.activation(pslice, sp, AF.Exp,
                                             scale=EXPSCALE,
                                             bias=bqm[:, jt:jt + 1],
                                             accum_out=col)
                    # v production rides the roomier 2079ns accum-exp windows
                    # of the last i-chunk; vts is complete well before the
                    # first vss scaling at jt==4
                    if c == NIC - 1 and jt == 0:
                        v_mms(0)
                    if c == NIC - 1 and jt == 1:
                        v_mms(4)
                    if c == NIC - 1:
                        nc.vector.tensor_reduce(
                            csum[:, jt:jt + 1], colsq[:, jt * 4:(jt + 1) * 4],
                            mybir.AxisListType.X, mybir.AluOpType.add)
                        nc.vector.reciprocal(ginv[:, jt:jt + 1],
                                             csum[:, jt:jt + 1])
                        nc.vector.tensor_scalar_mul(
                            ginv[:, jt:jt + 1], ginv[:, jt:jt + 1], gsc)
                        if jt >= 4:
                            t = jt - 4
                            for jm in range(2):
                                nc.vector.tensor_scalar_mul(
                                    vss[t][:, jm, :], vts[t][:, jm, :],
                                    ginv[:, t + 4 * jm:t + 4 * jm + 1])

            # ---- PV phase: u''^T[c, i] partials, drained fp8 to pbuf ----
            drain_engs = [nc.gpsimd, nc.scalar, nc.vector]
            dri = [0]
            for g in range(4):
                stg = strm.tile([128, 2, 2, NL], f8, tag="stg", bufs=2,
                                name=f"stg{g}")  # [cp, r', ct, i']
                for ct in range(2):
                    pv = psum.tile([128, 2, NL], f32, tag="S", bufs=2,
                                   name="pv")    # [cp, r', i']
                    for sub in range(4):
                        ic = g * 4 + sub
                        for t in range(NPAIR):
                            nc.tensor.matmul(
                                pv[:, sub // 2, (sub % 2) * 512:(sub % 2 + 1) * 512],
                                lhsT=vss[t][:, :, ct * 128:(ct + 1) * 128],
                                rhs=pts[t][:, :, ic * 512:(ic + 1) * 512],
                                start=(t == 0), stop=(t == NPAIR - 1),
                                perf_mode=DR,
                            )
                    for rr in range(2):
                        e = drain_engs[dri[0] % 3]
                        dri[0] += 1
                        if e is nc.scalar:
                            nc.scalar.copy(stg[:, rr, ct, :], pv[:, rr, :])
                        else:
                            e.tensor_scalar_add(stg[:, rr, ct, :],
                                                pv[:, rr, :], 0.0)
                    gb = 2 * 2 * 128 * NL
                    nc.sync.dma_start(
                        out=pbuf[g * gb:(g + 1) * gb].rearrange(
                            "(r p c i) -> p r c i", r=2, p=128, c=2)[:, :, ct, :],
                        in_=stg[:, :, ct, :],
                    )

            # ---- single collective: sum partials, scatter rows to owners ----
            nc.gpsimd.collective_compute(
                "ReduceScatter",
                mybir.AluOpType.add,
                replica_groups=RG,
                ins=[pbuf[:]],
                outs=[rsout[:]],
            )

            # ---- final: y = x@(a W1W2) + u''@(8 W2)/(8 VS) + b'' ----
            # x-term + PE warm-up dummies run DURING the ReduceScatter
            fx = psum.tile([128, 2048], f32, tag="S", bufs=2, name="fx")
            for it in range(8):
                for u in range(4):
                    nc.tensor.matmul(
                        fx[:, it * 256:(it + 1) * 256],
                        lhsT=xtlb[:, u, it * 128:(it + 1) * 128],
                        rhs=w12t[:, u, :],
                        start=(u == 0), stop=(u == 3),
                    )
            for it in range(8):
                e = [nc.vector, nc.gpsimd][it % 2]
                e.tensor_add(xyt[:, it, :],
                             fx[:, it * 256:(it + 1) * 256], b2t)
            nc.sync.dma_start(
                out=uts,
                in_=rsout[:].rearrange("(p c i) -> p c i", p=128, c=2),
            )
            # keep the PE p-state at full clock across the collective (slow
            # f32 matmuls into a scratch psum tile, no data deps on the RS);
            # the last few shrink so the overshoot past uts-arrival is small
            fdum = psum.tile([128, 2048], f32, tag="S", bufs=2, name="fdum")
            for d in range(NDUMMY):
                nc.tensor.matmul(
                    fdum[:, 0:512],
                    lhsT=bpt[:, 0:128],
                    rhs=bpt[:, 0:512],
                    start=True, stop=True,
                )
            for d in range(NDSMALL):
                nc.tensor.matmul(
                    fdum[:, 0:128],
                    lhsT=bpt[:, 0:128],
                    rhs=bpt[:, 0:128],
                    start=True, stop=True,
                )
            # one psum BANK per it-slice (512-f32 stride): the yst add that
            # reads slice it must not share a bank with the it+1 matmul or
            # the bank-granular WAR check serializes the projection stream
            fya = psum.tile([128, 2048], f32, tag="S", bufs=2, name="fya")
            fyb = psum.tile([128, 2048], f32, tag="S", bufs=2, name="fyb")
            # all 8 projection matmuls first (no add interleaved in between:
            # a psum-tile read by an add would stall later matmul writes),
            # then the adds stream on two engines, then the y stores
            def fsl(it):
                return [fya, fyb][it % 2][:, (it // 2) * 512:
                                          (it // 2) * 512 + 256]
            for it in range(8):
                nc.tensor.matmul(
                    fsl(it),
                    lhsT=uts[:, :, it * 128:(it + 1) * 128],
                    rhs=w2t[:, :, :],
                    start=True, stop=True, perf_mode=DR,
                )
            for b in range(4):
                yst = strm.tile([128, 2, C], f32, tag="yst", bufs=4,
                                name=f"yst{b}")
                for qq in range(2):
                    it = b * 2 + qq
                    e = [nc.vector, nc.gpsimd][it % 2]
                    e.scalar_tensor_tensor(
                        yst[:, qq, :],
                        fsl(it),
                        1.0 / (W2S * VS),
                        xyt[:, it, :],
                        op0=mybir.AluOpType.mult,
                        op1=mybir.AluOpType.add,
                    )
                nc.sync.dma_start(
                    out=y[b * 256:(b + 1) * 256, :].rearrange(
                        "(a p) c -> p a c", a=2, p=128),
                    in_=yst,
                )

    nc.finalize()
    return nc


def _get_nc():
    if "nc" not in _CACHED:
        _CACHED["nc"] = _build()
    return _CACHED["nc"]


def _prep_in_maps(x, W1, b1, Wq, bq, Wk, bk, Wv, bv, a, W2, b2):
    f32 = np.float32
    x = np.asarray(x, f32)
    W1 = np.asarray(W1, f32)
    b1 = np.asarray(b1, f32)
    av = f32(np.asarray(a, f32).reshape(-1)[0])

    def fold(Wx, bx):
        Wx, bx = np.asarray(Wx, f32), np.asarray(bx, f32)
        return W1 @ Wx, b1 @ Wx + bx

    Wqf, bqf = fold(Wq, bq)
    Wkf, bkf = fold(Wk, bk)
    Wvf, bvf = fold(Wv, bv)
    W2_ = np.asarray(W2, f32)
    W12 = av * (W1 @ W2_)
    b12 = av * (b1 @ W2_) + np.asarray(b2, f32)

    def pack(W, dtype, kt):
        return np.ascontiguousarray(
            W.reshape(kt, 128, W.shape[1]).transpose(1, 0, 2)).astype(dtype)

    def bcol(v2):
        return np.ascontiguousarray(v2.reshape(2, 128).T)

    # Gram-folded attention-score factors (see module docstring):
    #   S_psum = (256 Wq' Wk'^T x_l^T)^T-contracted-with-x, exp bias =
    #   x_l.(64 Wk' bq)/1024 + (bk.bq)/16 per local row j
    G = f32(256.0) * (Wqf @ Wkf.T)              # [512 hin, 512 din]
    gb = f32(256.0) * (Wqf @ bkf)               # [512 hin]
    wb = f32(64.0) * (Wkf @ bqf)                # [512 din]
    cb = f32(np.dot(bkf, bqf) / 16.0)           # scalar

    bp = np.zeros((128, NBP), f32)
    bp[:, 4] = (f32(1.0) - av) * f32(VS)
    bp[:, 5:5 + C] = np.broadcast_to(bvf, (128, C))
    bp[:, 5 + C:5 + 2 * C] = np.broadcast_to(b12, (128, C))
    bp[:, 5 + 2 * C:9 + 2 * C] = gb.reshape(4, 128).T
    bp[:, 9 + 2 * C] = cb

    xT = np.ascontiguousarray(x.T.reshape(4, 128, N).transpose(1, 0, 2))
    xT8 = xT.astype(FP8)

    shared = {
        "xT8": xT8,
        "g8": pack(np.ascontiguousarray(G.T), FP8, 4),
        "wb8": np.ascontiguousarray(
            wb.reshape(4, 128).T)[:, :, None].astype(FP8),
        "wv8": pack(Wvf, FP8, 4),
        "w12b": pack(W12, BF16, 4),
        "w2f": pack(W2_ * f32(W2S), FP8, 2),
        "bpk": bp,
    }
    maps = []
    for r in range(NCORES):
        sl = slice(r * NL, (r + 1) * NL)
        maps.append({
            **shared,
            "xTl8": np.ascontiguousarray(xT8[:, :, sl]),
            "xTlb": np.ascontiguousarray(xT[:, :, sl]).astype(BF16),
        })
    return maps


def kernel(**inputs) -> np.ndarray:
    from concourse.bass_utils import run_bass_kernel_spmd

    nc = _get_nc()
    in_maps = _prep_in_maps(**inputs)
    res = run_bass_kernel_spmd(nc, in_maps, list(range(NCORES)))
    return np.ascontiguousarray(
        np.concatenate(
            [res.results[r]["y"] for r in range(NCORES)], axis=0
        ).astype(np.float32)
    )
